# revision 47
# baseline (speedup 1.0000x reference)
"""Trainium2 Bass kernel for nn_AttentionModel: single-head attention with
vocab-sized input/output projections, tensor-parallel across 8 NeuronCores.

Math (reference):
    Q = x @ Wq + bq ; K = x @ Wk + bk ; V = x @ Wv + bv        [S, E]
    scores = Q @ K^T / sqrt(E)                                  [S, S]
    out = softmax(scores) @ V @ Wo + bo                         [S, VOCAB]

Sharding: vocab dim (50257, padded to 8*6400) split across 8 cores.
  Pass 1: per-core partial [K|V] = x_c @ [Wk|Wv]_c  (one sweep over x,
          both projections share each x tile as the stationary operand)
          -> AllReduce in row chunks, overlapped with the rest of the sweep.
  Pass 2: per-core partial Q = x_c @ Wq_c(scaled)   -> ReduceScatter
          (each core ends with its own 256-query slice of Q).  K^T on-chip
          transposes + V readback are interleaved into this sweep.
  Phase C: scores^T = K @ Q_s^T, exp (no max subtraction needed: scores are
           ~N(0,1)), unnormalized ctx^T = V^T @ exp^T, denominators via
           ones-matmul.  ctx^T + denom rows -> AllGather in 2 query-halves.
  Phase D: out_c = ctx @ Wo_c, normalized by 1/denom at PSUM eviction;
           h=0 seq tiles first so they only wait on the first AllGather.
All matmuls run bf16 inputs with fp32 PSUM accumulation.  1/sqrt(E) is folded
into Wq host-side; Q/K/V biases ride a ones-row in the padded vocab dim.
Output is written bf16 and upcast host-side (rel-err budget is 2e-2).

x host layout: [ST*128, KT*128] where row st*128+p, col k*128+s' holds
xT[k*128+p, st*128+s'] -- so each seq tile's slab is one contiguous
12.8KB-per-partition DMA, and each k-slice is a [128,128] stationary tile.
"""

import sys

if "/opt/trn_rl_repo" not in sys.path:
    sys.path.insert(0, "/opt/trn_rl_repo")

import numpy as np
import ml_dtypes

import concourse.bass as bass
import concourse.tile as tile
from concourse import bacc, mybir
from concourse import bass_utils
from concourse.masks import make_identity

BF16 = mybir.dt.bfloat16
F32 = mybir.dt.float32
NP_BF16 = ml_dtypes.bfloat16


class Cfg:
    def __init__(self, S=2048, E=768, VS=6400, n_cores=8, vocab=50257):
        assert S % 512 == 0 and E % 128 == 0 and VS % 128 == 0
        self.S = S  # full sequence
        self.E = E  # embed dim
        self.VS = VS  # padded vocab rows per core
        self.n_cores = n_cores
        self.vocab = vocab
        self.ST = S // 128  # seq tiles
        self.ET = E // 128  # embed tiles
        self.KT = VS // 128  # contraction (vocab) tiles per core
        self.QS = S // n_cores  # queries per core
        assert self.QS % 128 == 0
        self.QT = self.QS // 128
        # AllReduce row-chunks for pass 1 (in seq tiles).  Equal chunks in
        # separate DRAM tensors: no false W-after-R hazards between a chunk's
        # collective and later tiles' evictions.  Sized so each chunk's
        # collective finishes before the next is emitted -- a queued
        # collective blocks the Pool sequencer, which stalls the DMA sync
        # chain and ultimately the PE.
        self.ar_chunks = [(0, 4), (4, 8), (8, 12), (12, 15), (15, 16)]
        # phase D vocab chunks (over this core's VS output columns)
        self.nch = [(i * 512, min(512, VS - i * 512)) for i in range((VS + 511) // 512)]


FULL = Cfg()


def build_nc(cfg: Cfg, reps: int = 1, emulate_cc: bool = False):
    S, E, VS = cfg.S, cfg.E, cfg.VS
    ST, ET, KT, QS, QT = cfg.ST, cfg.ET, cfg.KT, cfg.QS, cfg.QT
    KV = 2 * E  # fused K|V column width
    RG = [list(range(cfg.n_cores))]

    nc = bacc.Bacc(None, target_bir_lowering=False, num_devices=cfg.n_cores)

    x4 = nc.dram_tensor("xs4", [ST * 128, KT * 128], BF16, kind="ExternalInput")
    wkv = nc.dram_tensor("wkv", [VS, KV], BF16, kind="ExternalInput")
    wq = nc.dram_tensor("wq", [VS, E], BF16, kind="ExternalInput")
    wo = nc.dram_tensor("wo", [E, VS], BF16, kind="ExternalInput")
    out = nc.dram_tensor("out", [S, VS], BF16, kind="ExternalOutput")

    x4_t = x4.ap().rearrange("(st p) f -> p st f", p=128)
    wkv_t = wkv.ap().rearrange("(kt p) e -> p kt e", p=128)
    wq_t = wq.ap().rearrange("(kt p) e -> p kt e", p=128)
    wo_t = wo.ap().rearrange("(et p) v -> p et v", p=128)

    # internal DRAM for collectives (one tensor pair per kv AR chunk so the
    # collective's read of chunk i never hazards later chunks' writes)
    kv_in_c = [
        nc.dram_tensor(f"kv_in_{i}", [(c1 - c0) * 128, KV], BF16)
        for i, (c0, c1) in enumerate(cfg.ar_chunks)
    ]
    kv_out_c = [
        nc.dram_tensor(f"kv_out_{i}", [(c1 - c0) * 128, KV], BF16, addr_space="Shared")
        for i, (c0, c1) in enumerate(cfg.ar_chunks)
    ]
    q_in_h = [
        nc.dram_tensor(f"q_in_{h}", [S // 2, E], BF16) for h in range(QT)
    ]
    q_out_h = [
        nc.dram_tensor(f"q_out_{h}", [128, E], BF16) for h in range(QT)
    ]
    ctx_in_h = [
        nc.dram_tensor(f"ctx_in_{h}", [E + 2, 128], BF16) for h in range(QT)
    ]
    ctx_out_h = [
        nc.dram_tensor(
            f"ctx_out_{h}", [cfg.n_cores * (E + 2), 128], BF16, addr_space="Shared"
        )
        for h in range(QT)
    ]

    def do_cc(kind, in_ap, out_ap, nrows_in=None, nrows_out=None):
        if not emulate_cc:
            op = (
                mybir.AluOpType.bypass
                if kind == "AllGather"
                else mybir.AluOpType.add
            )
            nc.gpsimd.collective_compute(
                kind,
                op,
                replica_groups=RG,
                ins=[in_ap.opt()],
                outs=[out_ap.opt()],
            )
            return
        # single-core emulation with plain DMA (preserves deps for sims)
        if kind == "AllReduce":
            nc.sync.dma_start(out=out_ap, in_=in_ap)
        elif kind == "ReduceScatter":
            nc.sync.dma_start(out=out_ap, in_=in_ap[0:nrows_out, :])
        elif kind == "AllGather":
            for c in range(cfg.n_cores):
                nc.sync.dma_start(
                    out=out_ap[c * nrows_in : (c + 1) * nrows_in, :], in_=in_ap
                )

    with tile.TileContext(nc) as tc:
        const = tc.alloc_tile_pool(name="const", bufs=1)
        id128 = const.tile([128, 128], BF16)
        make_identity(nc, id128)
        ones = const.tile([128, 1], BF16)
        nc.vector.memset(ones, 1.0)

        for rep in range(reps):
            # ---------------- Pass 1: partial [K|V] = x_c @ [Wk|Wv]_c --------
            xio = tc.alloc_tile_pool(name="xio", bufs=1)
            wkvp = tc.alloc_tile_pool(name="wkvp", bufs=1, side="right")
            wkv_sb = wkvp.tile([128, KT, KV], BF16)
            ps1 = tc.alloc_tile_pool(name="ps1", bufs=8, space="PSUM")

            # m=0 and m=1 run jointly: the wkv weight stream (~58us) outpaces
            # one tile's compute (~32us), two tiles' compute matches it.
            xts = {}
            hk = KT // 2
            groups = [[0, 1]] + [[m] for m in range(2, ST)]
            for grp in groups:
                if grp[0] == 0:
                    # first couple of wkv slices ahead of the x tiles: the
                    # very first matmul needs wkv[0], which otherwise queues
                    # behind ~10us of x transfers
                    for k in range(2):
                        nc.sync.dma_start(out=wkv_sb[:, k, :], in_=wkv_t[:, k, :])
                for m in grp:
                    xta = xio.tile(
                        [128, hk * 128], BF16, tag="x", bufs=4, name=f"xa{m}"
                    )
                    nc.sync.dma_start(out=xta, in_=x4_t[:, m, 0 : hk * 128])
                    xtb = xio.tile(
                        [128, hk * 128], BF16, tag="x", bufs=4, name=f"xb{m}"
                    )
                    nc.sync.dma_start(out=xtb, in_=x4_t[:, m, hk * 128 :])
                    xts[m] = (xta, xtb)
                pss = {
                    m: [
                        ps1.tile([128, 512], F32, name=f"p1_{m}_{j}", tag="p1")
                        for j in range(3)
                    ]
                    for m in grp
                }
                for k in range(KT):
                    if grp[0] == 0 and k >= 2:
                        nc.sync.dma_start(out=wkv_sb[:, k, :], in_=wkv_t[:, k, :])
                    for m in grp:
                        xh = xts[m][0] if k < hk else xts[m][1]
                        lhs = xh[:, (k % hk) * 128 : (k % hk + 1) * 128]
                        for j in range(3):
                            nc.tensor.matmul(
                                pss[m][j],
                                lhsT=lhs,
                                rhs=wkv_sb[:, k, j * 512 : (j + 1) * 512],
                                start=(k == 0),
                                stop=(k == KT - 1),
                            )
                for m in grp:
                    stg = xio.tile([128, KV], BF16, tag="st1", bufs=5, name=f"sg{m}")
                    nc.vector.tensor_copy(stg[:, 0:512], pss[m][0])
                    nc.scalar.activation(
                        out=stg[:, 512:1024],
                        in_=pss[m][1],
                        func=mybir.ActivationFunctionType.Copy,
                    )
                    nc.vector.tensor_copy(stg[:, 1024:1536], pss[m][2])
                    for ci, (c0, c1) in enumerate(cfg.ar_chunks):
                        if c0 <= m < c1:
                            # eviction writes ride the scalar queue so they
                            # never head-block the x/w loads on SP
                            nc.scalar.dma_start(
                                out=kv_in_c[ci][
                                    (m - c0) * 128 : (m - c0 + 1) * 128, :
                                ],
                                in_=stg,
                            )
                        if m == c1 - 1:
                            do_cc(
                                "AllReduce",
                                kv_in_c[ci].ap(),
                                kv_out_c[ci].ap(),
                            )
            ps1.release()
            wkvp.release()

            # ---------------- Pass 2: partial Q = x_c @ Wq_c -----------------
            # Query ownership is split: core c owns seq rows [128c,128c+128)
            # (half A, reduce-scattered from rows 0:1024 once tiles 0..7 are
            # done, mid-sweep) and rows [1024+128c, ...) (half B, RS at the
            # end).  This hides most of the Q reduce-scatter and lets each
            # half's attention -> AllGather -> out-projection pipeline.
            # K^T transposes + V readback also interleave with the sweep.
            attn = tc.alloc_tile_pool(name="attn", bufs=1, side="right")
            kT_sb = attn.tile([128, ET, S], BF16)
            v_sb = attn.tile([128, ST, E], BF16)
            qT_sb = attn.tile([128, ET, QS], BF16)
            expT_sb = attn.tile([128, ST, QS], BF16)
            wqp = tc.alloc_tile_pool(name="wqp", bufs=1, side="right")
            wq_sb = wqp.tile([128, KT, E], BF16)
            psT = tc.alloc_tile_pool(name="psT", bufs=2, space="PSUM")
            ps2 = tc.alloc_tile_pool(name="ps2", bufs=3, space="PSUM")

            def qt_transposes(h):
                q_sb = xio.tile([128, E], BF16, tag="qsb", bufs=2, name=f"qsb{h}")
                nc.gpsimd.dma_start(out=q_sb, in_=q_out_h[h].ap())
                for et in range(ET):
                    ps_t = psT.tile([128, 128], BF16, tag="pt")
                    nc.tensor.transpose(
                        ps_t, q_sb[:, et * 128 : (et + 1) * 128], id128
                    )
                    nc.vector.tensor_copy(
                        qT_sb[:, et, h * 128 : (h + 1) * 128], ps_t
                    )

            # tiles 14,15 first (their x tiles are still resident from
            # pass 1), then the rest of half B (8..13) so RS-B fires
            # mid-sweep, then half A (0..7) feeding RS-A at the end
            p2_order = [ST - 2, ST - 1] + list(range(ST // 2, ST - 2)) + list(
                range(0, ST // 2)
            )
            for mi, m in enumerate(p2_order):
                if m >= ST - 2:
                    xta, xtb = xts[m]
                else:
                    xta = xio.tile(
                        [128, hk * 128], BF16, tag="x", bufs=4, name=f"xqa{m}"
                    )
                    nc.sync.dma_start(out=xta, in_=x4_t[:, m, 0 : hk * 128])
                    xtb = xio.tile(
                        [128, hk * 128], BF16, tag="x", bufs=4, name=f"xqb{m}"
                    )
                    nc.sync.dma_start(out=xtb, in_=x4_t[:, m, hk * 128 :])
                psa = ps2.tile([128, 512], F32, name=f"p2a_{m}", tag="p2a")
                psb = ps2.tile([128, 256], F32, name=f"p2b_{m}", tag="p2b")
                for k in range(KT):
                    if mi == 0:
                        nc.sync.dma_start(out=wq_sb[:, k, :], in_=wq_t[:, k, :])
                    xh = xta if k < hk else xtb
                    lhs = xh[:, (k % hk) * 128 : (k % hk + 1) * 128]
                    nc.tensor.matmul(
                        psa,
                        lhsT=lhs,
                        rhs=wq_sb[:, k, 0:512],
                        start=(k == 0),
                        stop=(k == KT - 1),
                    )
                    nc.tensor.matmul(
                        psb,
                        lhsT=lhs,
                        rhs=wq_sb[:, k, 512:768],
                        start=(k == 0),
                        stop=(k == KT - 1),
                    )
                stg = xio.tile([128, E], BF16, tag="st2", bufs=6, name=f"sq{m}")
                nc.vector.tensor_copy(stg[:, 0:512], psa)
                nc.scalar.activation(
                    out=stg[:, 512:768],
                    in_=psb,
                    func=mybir.ActivationFunctionType.Copy,
                )
                hq, mq = (0, m) if m < ST // 2 else (1, m - ST // 2)
                nc.scalar.dma_start(
                    out=q_in_h[hq][mq * 128 : (mq + 1) * 128, :], in_=stg
                )
                if m == ST - 3:
                    # all of half B evicted: reduce-scatter it mid-sweep
                    do_cc(
                        "ReduceScatter",
                        q_in_h[1].ap(),
                        q_out_h[1].ap(),
                        nrows_out=128,
                    )
                if mi == 11:
                    qt_transposes(1)
                # interleaved K^T/V fetch for seq tile mi (ascending): its AR
                # chunk completed long before, and the 6 transposes hide in
                # this iteration's matmul stream
                tb = mi
                ci = next(
                    i for i, (c0, c1) in enumerate(cfg.ar_chunks) if c0 <= tb < c1
                )
                c0 = cfg.ar_chunks[ci][0]
                krow = xio.tile([128, E], BF16, tag="krow", bufs=3)
                nc.gpsimd.dma_start(
                    out=krow,
                    in_=kv_out_c[ci][(tb - c0) * 128 : (tb - c0 + 1) * 128, 0:E],
                )
                nc.gpsimd.dma_start(
                    out=v_sb[:, tb, :],
                    in_=kv_out_c[ci][(tb - c0) * 128 : (tb - c0 + 1) * 128, E:KV],
                )
                for et in range(ET):
                    ps_t = psT.tile([128, 128], BF16, tag="pt")
                    nc.tensor.transpose(
                        ps_t, krow[:, et * 128 : (et + 1) * 128], id128
                    )
                    nc.vector.tensor_copy(
                        kT_sb[:, et, tb * 128 : (tb + 1) * 128], ps_t
                    )
            do_cc(
                "ReduceScatter", q_in_h[0].ap(), q_out_h[0].ap(), nrows_out=128
            )
            ps2.release()
            wqp.release()

            # ---------------- Phase C: attention, one query-half at a time ---
            # wo loads trickle during phase C / AllGather window.
            wop = tc.alloc_tile_pool(name="wop", bufs=1, side="right")
            wo_sb = wop.tile([128, ET, VS], BF16)
            blk = E + 2
            ctxp = tc.alloc_tile_pool(name="ctxp", bufs=1, side="right")
            d_tiles = {}

            def emit_d_loads(ms):
                h, c_blk = (0, ms) if ms < ST // 2 else (1, ms - ST // 2)
                r0 = blk * c_blk
                ctxT_bf = ctxp.tile(
                    [128, ET, 128], BF16, tag="ctx", bufs=6, name=f"cx{ms}"
                )
                nc.gpsimd.dma_start(
                    out=ctxT_bf,
                    in_=ctx_out_h[h][r0 : r0 + E, :].rearrange(
                        "(et p) q -> p et q", p=128
                    ),
                )
                dhl = ctxp.tile([128, 2], BF16, tag="dh", bufs=6, name=f"dh{ms}")
                nc.gpsimd.dma_start(
                    out=dhl,
                    in_=ctx_out_h[h][r0 + E : r0 + E + 2, :].rearrange(
                        "two q -> q two"
                    ),
                )
                d_tiles[ms] = (ctxT_bf, dhl)

            cstage = tc.alloc_tile_pool(name="cstage", bufs=1)
            psC = tc.alloc_tile_pool(name="psC", bufs=1, space="PSUM")
            for et in range(ET):
                nc.sync.dma_start(out=wo_sb[:, et, :], in_=wo_t[:, et, :])
            for h in (1, 0):
                if h == 0:
                    qt_transposes(0)
                # scores^T tiles + exp for this half
                for mk in range(ST):
                    ps_s = psC.tile([128, 128], F32, tag="ps_s", bufs=3)
                    for et in range(ET):
                        nc.tensor.matmul(
                            ps_s,
                            lhsT=kT_sb[:, et, mk * 128 : (mk + 1) * 128],
                            rhs=qT_sb[:, et, h * 128 : (h + 1) * 128],
                            start=(et == 0),
                            stop=(et == ET - 1),
                        )
                    nc.scalar.activation(
                        out=expT_sb[:, mk, h * 128 : (h + 1) * 128],
                        in_=ps_s,
                        func=mybir.ActivationFunctionType.Exp,
                    )
                # denominators via ones-matmul (after the scores loop so the
                # in-order PE never waits on each exp)
                ps_d = psC.tile([1, 128], F32, tag="ps_d", bufs=1)
                for mk in range(ST):
                    nc.tensor.matmul(
                        ps_d,
                        lhsT=ones,
                        rhs=expT_sb[:, mk, h * 128 : (h + 1) * 128],
                        start=(mk == 0),
                        stop=(mk == ST - 1),
                    )
                dhi = cstage.tile([1, 128], BF16, tag="d1", bufs=2)
                nc.vector.tensor_copy(dhi, ps_d)
                dhi_f = cstage.tile([1, 128], F32, tag="d2", bufs=2)
                nc.vector.tensor_copy(dhi_f, dhi)
                dlo_f = cstage.tile([1, 128], F32, tag="d3", bufs=2)
                nc.vector.tensor_sub(dlo_f, ps_d, dhi_f)
                dlo = cstage.tile([1, 128], BF16, tag="d4", bufs=2)
                nc.vector.tensor_copy(dlo, dlo_f)
                nc.scalar.dma_start(out=ctx_in_h[h][E : E + 1, :], in_=dhi)
                nc.scalar.dma_start(out=ctx_in_h[h][E + 1 : E + 2, :], in_=dlo)
                # unnormalized ctx^T = V^T @ exp^T for this half
                for et in range(ET):
                    ps_c = psC.tile([128, 128], F32, tag="ps_c", bufs=2)
                    for mk in range(ST):
                        nc.tensor.matmul(
                            ps_c,
                            lhsT=v_sb[:, mk, et * 128 : (et + 1) * 128],
                            rhs=expT_sb[:, mk, h * 128 : (h + 1) * 128],
                            start=(mk == 0),
                            stop=(mk == ST - 1),
                        )
                    cstg = cstage.tile([128, 128], BF16, tag="cst", bufs=4)
                    nc.vector.tensor_copy(cstg, ps_c)
                    nc.scalar.dma_start(
                        out=ctx_in_h[h][et * 128 : (et + 1) * 128, :], in_=cstg
                    )
                do_cc(
                    "AllGather",
                    ctx_in_h[h].ap(),
                    ctx_out_h[h].ap(),
                    nrows_in=E + 2,
                )
                if h == 1:
                    # pre-issue the first half-B out tiles' ctx loads while
                    # the Pool queue is unblocked (the second AllGather's
                    # dispatch will stall it on the other half's evictions)
                    for ms in range(ST // 2, ST // 2 + 6):
                        emit_d_loads(ms)
            psC.release()
            psT.release()
            cstage.release()
            xio.release()
            osb = tc.alloc_tile_pool(name="osb", bufs=1, side="right")

            # ---------------- Phase D: out_c = ctx @ Wo_c / denom ------------
            # seq tile ms < 8 holds core ms's half-A queries (AllGather A);
            # ms >= 8 holds core (ms-8)'s half-B queries (AllGather B).
            psD = tc.alloc_tile_pool(name="psD", bufs=8, space="PSUM")
            for ms in list(range(ST // 2, ST)) + list(range(ST // 2)):
                if ms not in d_tiles:
                    emit_d_loads(ms)
                ctxT_bf, dhl = d_tiles.pop(ms)
                dsum = ctxp.tile([128, 1], F32, tag="ds", bufs=3)
                nc.vector.tensor_add(dsum, dhl[:, 0:1], dhl[:, 1:2])
                recip = ctxp.tile([128, 1], F32, tag="rc", bufs=3)
                nc.vector.reciprocal(recip, dsum)
                ost = osb.tile([128, VS], BF16, tag="ost", bufs=3)
                half = 3072
                for ni, (n0, nsz) in enumerate(cfg.nch):
                    ps_o = psD.tile([128, 512], F32, tag="pd")
                    for et in range(ET):
                        nc.tensor.matmul(
                            ps_o[:, :nsz],
                            lhsT=ctxT_bf[:, et, :],
                            rhs=wo_sb[:, et, n0 : n0 + nsz],
                            start=(et == 0),
                            stop=(et == ET - 1),
                        )
                    nc.vector.tensor_scalar_mul(
                        ost[:, n0 : n0 + nsz], ps_o[:, :nsz], recip
                    )
                    if n0 + nsz == half:
                        nc.scalar.dma_start(
                            out=out[ms * 128 : (ms + 1) * 128, 0:half],
                            in_=ost[:, 0:half],
                        )
                nc.scalar.dma_start(
                    out=out[ms * 128 : (ms + 1) * 128, half:VS],
                    in_=ost[:, half:VS],
                )
            psD.release()
            osb.release()
            ctxp.release()
            wop.release()
            attn.release()

        const.release()

    nc.compile()
    return nc


def _shard_bounds(cfg: Cfg):
    base = cfg.vocab // cfg.n_cores
    rem = cfg.vocab % cfg.n_cores
    sizes = [base + (1 if c < rem else 0) for c in range(cfg.n_cores)]
    starts = [sum(sizes[:c]) for c in range(cfg.n_cores)]
    return starts, sizes


def prepare_inputs(cfg: Cfg, x, Wq, bq, Wk, bk, Wv, bv, Wo):
    """Host-side shard/pad/cast. Returns in_maps for run_bass_kernel_spmd."""
    S, E, VS, N = cfg.S, cfg.E, cfg.VS, cfg.n_cores
    KT, ST = cfg.KT, cfg.ST
    inv = np.float32(1.0 / np.sqrt(E))
    xT = np.ascontiguousarray(x.reshape(S, -1).T.astype(np.float32)).astype(NP_BF16)
    Wq_s = (Wq.astype(np.float32) * inv).astype(NP_BF16)
    Wk_s = Wk.astype(np.float32).astype(NP_BF16)
    Wv_s = Wv.astype(np.float32).astype(NP_BF16)
    Wo_s = Wo.astype(np.float32).astype(NP_BF16)
    bq_s = (bq.astype(np.float32) * inv / N).astype(np.float32)
    bk_s = (bk.astype(np.float32) / N).astype(np.float32)
    bv_s = (bv.astype(np.float32) / N).astype(np.float32)

    starts, sizes = _shard_bounds(cfg)
    in_maps = []
    for c in range(N):
        s0, rv = starts[c], sizes[c]
        assert rv <= VS - 1, "need a free padded row for the bias/ones row"
        xs = np.zeros((VS, S), dtype=NP_BF16)
        xs[:rv] = xT[s0 : s0 + rv]
        xs[VS - 1] = NP_BF16(1.0)
        # re-layout: [VS, S] -> [ST*128, KT*128] with
        # row st*128+p, col k*128+s' = xs[k*128+p, st*128+s']
        x4c = np.ascontiguousarray(
            xs.reshape(KT, 128, ST, 128).transpose(2, 1, 0, 3)
        ).reshape(ST * 128, KT * 128)
        wkvc = np.zeros((VS, 2 * E), dtype=NP_BF16)
        wkvc[:rv, 0:E] = Wk_s[s0 : s0 + rv]
        wkvc[VS - 1, 0:E] = bk_s.astype(NP_BF16)
        wkvc[:rv, E : 2 * E] = Wv_s[s0 : s0 + rv]
        wkvc[VS - 1, E : 2 * E] = bv_s.astype(NP_BF16)
        wqc = np.zeros((VS, E), dtype=NP_BF16)
        wqc[:rv] = Wq_s[s0 : s0 + rv]
        wqc[VS - 1] = bq_s.astype(NP_BF16)
        woc = np.zeros((E, VS), dtype=NP_BF16)
        woc[:, :rv] = Wo_s[:, s0 : s0 + rv]
        in_maps.append({"xs4": x4c, "wkv": wkvc, "wq": wqc, "wo": woc})
    return in_maps


def assemble_output(cfg: Cfg, results, bo):
    starts, sizes = _shard_bounds(cfg)
    parts = [
        results[c]["out"][:, : sizes[c]].astype(np.float32)
        for c in range(cfg.n_cores)
    ]
    full = np.concatenate(parts, axis=1)
    full = full + bo.astype(np.float32)[None, :]
    return full[None].astype(np.float32)


_NC_CACHE = {}


def _get_nc(cfg: Cfg):
    key = (cfg.S, cfg.E, cfg.VS, cfg.n_cores)
    if key not in _NC_CACHE:
        _NC_CACHE[key] = build_nc(cfg)
    return _NC_CACHE[key]


def kernel(x, Wq, bq, Wk, bk, Wv, bv, Wo, bo):
    cfg = FULL
    x = np.asarray(x)
    in_maps = prepare_inputs(
        cfg,
        x,
        np.asarray(Wq),
        np.asarray(bq),
        np.asarray(Wk),
        np.asarray(bk),
        np.asarray(Wv),
        np.asarray(bv),
        np.asarray(Wo),
    )
    nc = _get_nc(cfg)
    res = bass_utils.run_bass_kernel_spmd(
        nc, in_maps, core_ids=list(range(cfg.n_cores))
    )
    return assemble_output(cfg, res.results, np.asarray(bo))
